# revision 12
# baseline (speedup 1.0000x reference)
"""VisionZip text-aware token-selection kernel for Trainium2 (Bass/Tile), v2.

Contract: kernel(**inputs) takes FULL inputs (B=32) and returns the FULL
output [32, 65, 1024]. Pure data-parallel over 8 NeuronCores (4 samples each).

v2 changes vs baseline:
  - hidden passed as single bf16 copy (fp16 runs at 2 cycles/row on the PE;
    bf16 runs at 1) -> big matmul phase ~4x faster, hidden DMA halved.
  - 1/cnt folded into the C matrix (C_ctx = itgt + eqm/cnt), so the PSUM
    result is final: no per-output recip multiply, fewer count matmuls.
  - affine score trick: rank order of 0.5*z(Sd)+0.5*z(cos) equals rank order
    of a*Sd + b*cos with a=0.5/(std_sd+eps), b=0.5/(std_cos+eps) (per-sample
    constants cancel) -> no mean-subtraction passes over [*,576].
  - column-major score pipeline: attn CLS-row and metric are host-transposed
    to [token(128p), chunk, ...] so Sd is one X-reduce and all selection
    tensors stay in the [128, 5, BC] layout; sums/sumsqs via ones-matmuls.
  - rank compare+accum ops read the broadcast scores directly from PSUM and
    are split across Vector and GpSimd.
  - sim/Tn matmuls batched into single PSUM tiles per chunk; rmx/eqm read
    PSUM directly (no sim copies).
  - hidden DMAs on the scalar-engine HWDGE ring, small inputs on the sync
    ring (parallel input streams); outputs also on the scalar ring.
"""
import numpy as np

import sys
if '/opt/trn_rl_repo' not in sys.path:
    sys.path.insert(0, '/opt/trn_rl_repo')

import concourse.bacc as bacc
import concourse.tile as tile
from concourse import mybir
from concourse.bass_utils import run_bass_kernel_spmd

F32 = mybir.dt.float32
F32R = mybir.dt.float32r
BF16 = mybir.dt.bfloat16
NPBF16 = mybir.dt.np(mybir.dt.bfloat16)
N_CORES = 8
BC = 4                      # samples per core
L = 577                     # tokens (incl CLS)
LPAD = 640
D = 1024
CK = 64
NH = 16
DOM = 54
NSEL = DOM + 1              # + CLS
CTX = 10
STEP = 52                   # (577-1-54) // 10
OUT_T = NSEL + CTX          # 65 output tokens
CHUNKS = [(0, 128), (128, 128), (256, 128), (384, 128), (512, 65)]
EQ = mybir.AluOpType
AF = mybir.ActivationFunctionType
AX = mybir.AxisListType


def _consts():
    c = {}
    c["c_iden"] = np.eye(128, dtype=np.float32)
    ut = (np.arange(128)[:, None] <= np.arange(128)[None, :]).astype(NPBF16)
    c["c_utb"] = ut
    c["c_onesb"] = np.ones((128, 128), NPBF16)
    c["c_onescol"] = np.ones((128, 1), np.float32)
    c["c_onescolb"] = np.ones((128, 1), NPBF16)
    c["c_ones1"] = np.ones((1, 128), np.float32)
    selbc = np.zeros((BC, BC * 128), np.float32)
    for s in range(BC):
        selbc[s, s * 128:(s + 1) * 128] = 1.0
    c["c_selbc"] = selbc
    c["c_iota55"] = (np.arange(NSEL) + 1.0).astype(np.float32).reshape(1, 1, NSEL) \
        .repeat(128, 0).copy()
    c["c_iota52"] = (-STEP * np.arange(CTX, dtype=np.float32)).reshape(1, 1, CTX) \
        .repeat(128, 0).copy()
    ii = np.zeros((128, 5, 1), np.float32)
    for ci, (off, _) in enumerate(CHUNKS):
        ii[:, ci, 0] = off + np.arange(128)
    c["c_iotaI"] = ii
    return c


_CONST_DTYPES = {"c_utb": BF16, "c_onesb": BF16, "c_onescolb": BF16}


def build_nc(stage=99):
    nc = bacc.Bacc("TRN2", target_bir_lowering=False, debug=False)

    attnT_d = nc.declare_dram_parameter("attnT", [128, 5, BC * NH], F32, isOutput=False)
    metricT_d = nc.declare_dram_parameter("metricT", [128, 5, BC, CK], F32, isOutput=False)
    text_d = nc.declare_dram_parameter("text", [BC, CK], F32, isOutput=False)
    hid_d = nc.declare_dram_parameter("hidb", [BC, L, D], BF16, isOutput=False)
    cshapes = {k: v.shape for k, v in _consts().items()}
    cdram = {k: nc.declare_dram_parameter(k, list(sh), _CONST_DTYPES.get(k, F32),
                                          isOutput=False)
             for k, sh in cshapes.items()}
    out_d = nc.declare_dram_parameter("out", [BC, OUT_T, D], F32, isOutput=True)

    with tile.TileContext(nc) as tc:
        with (
            tc.tile_pool(name="persist", bufs=1) as pp,
            tc.tile_pool(name="hidpool", bufs=1) as hp,
            tc.tile_pool(name="scratch", bufs=2) as sp,
            tc.tile_pool(name="ps_misc", bufs=2, space="PSUM") as ps_misc,
            tc.tile_pool(name="ps_bcs", bufs=1, space="PSUM") as ps_bcs,
            tc.tile_pool(name="ps_rr", bufs=1, space="PSUM") as ps_rr,
            tc.tile_pool(name="ps_out", bufs=2, space="PSUM") as ps_out,
        ):
            pools = (pp, hp, sp, ps_misc, ps_bcs, ps_rr, ps_out)
            _body(nc, stage, pools, attnT_d, metricT_d, text_d, hid_d,
                  cdram, cshapes, out_d)
    nc.compile()
    return nc


def _body(nc, stage, pools, attnT_d, metricT_d, text_d, hid_d,
          cdram, cshapes, out_d):
    pp, hp, sp, ps_misc, ps_bcs, ps_rr, ps_out = pools
    V = nc.vector
    A = nc.scalar
    G = nc.gpsimd
    T = nc.tensor
    DMA = nc.sync          # small inputs: sync-engine HWDGE ring
    DMA2 = nc.scalar       # hidden + outputs: scalar-engine HWDGE ring

    def dump(n):
        d = sp.tile([BC, 512], F32, tag="dump")
        V.memset(d[:], float(n))
        DMA.dma_start(out_d[:, 0, 0:512], d[:])

    # ---- input DMAs ----
    attnT = pp.tile([128, 5, BC * NH], F32, tag="attnT")
    DMA.dma_start(attnT[:], attnT_d[:])
    text_sb = pp.tile([BC, CK], F32, tag="text_sb")
    DMA.dma_start(text_sb[:], text_d[:])
    mt = pp.tile([128, 5, BC, CK], F32, tag="mt")
    DMA.dma_start(mt[:], metricT_d[:])
    csb = {}
    for k, sh in cshapes.items():
        t = pp.tile(list(sh), _CONST_DTYPES.get(k, F32), tag=k)
        DMA.dma_start(t[:], cdram[k][:])
        csb[k] = t
    # hidden (big): sync ring, queued after the small inputs
    hid = []
    for s in range(BC):
        row = []
        for ci, (off, k) in enumerate(CHUNKS):
            th = hp.tile([128, D], BF16, tag=f"h{s}_{ci}")
            DMA.dma_start(th[0:k, :], hid_d[s, off:off + k, :])
            row.append(th)
        hid.append(row)
    # prefetch the ACT Sqrt table during DMA wait
    dmt = sp.tile([1, 1], F32, tag="dmt")
    V.memset(dmt[:], 1.0)
    dmt2 = sp.tile([1, 1], F32, tag="dmt2")
    A.activation(dmt2[:], dmt[:], AF.Sqrt)

    if stage <= 1:
        return dump(1)

    # ---- text_n and its partition-broadcast ----
    tsc = sp.tile([BC, CK], F32, tag="tsc")
    V.tensor_mul(tsc[:], text_sb[:], text_sb[:])
    tss = pp.tile([BC, 1], F32, tag="tss")
    V.tensor_reduce(tss[:], tsc[:], axis=AX.X, op=EQ.add)
    tst = pp.tile([BC, 1], F32, tag="tst")
    A.activation(tst[:], tss[:], AF.Sqrt)
    trc = pp.tile([BC, 1], F32, tag="trc")
    V.reciprocal(trc[:], tst[:])
    textn = pp.tile([BC, CK], F32, tag="textn")
    V.tensor_scalar_mul(textn[:], text_sb[:], trc[:])
    tb_ps = ps_misc.tile([128, BC * CK], F32, tag="ps")
    for s in range(BC):
        T.matmul(tb_ps[:, s * CK:(s + 1) * CK],
                 csb["c_selbc"][:, s * 128:(s + 1) * 128],
                 textn[:, :], start=True, stop=True)
    textb = pp.tile([128, BC, CK], F32, tag="textb")
    A.copy(textb[:].rearrange("p s c -> p (s c)"), tb_ps[:, :])

    # ---- X tile: [128, 5, (sd s0..3 | cos s0..3 | sd^2 | cos^2)] ----
    X = pp.tile([128, 5, 16], F32, tag="X")
    # Sd: sum CLS-attention over 16 heads -> X[:, :, 0:4]
    V.tensor_reduce(X[:, :, 0:4], attnT[:].rearrange("p c (s h) -> p c s h", h=NH),
                    axis=AX.X, op=EQ.add)

    # ---- metric norms, mn, cos ----
    mn = pp.tile([128, 5, BC, CK], F32, tag="mn")
    rnorm = pp.tile([128, 5, BC, 1], F32, tag="rnorm")
    for ci, (off, k) in enumerate(CHUNKS):
        sq = sp.tile([128, BC, CK], F32, tag="sq")
        V.tensor_mul(sq[0:k], mt[0:k, ci], mt[0:k, ci])
        ssq = sp.tile([128, BC], F32, tag="ssq")
        V.tensor_reduce(ssq[0:k], sq[0:k], axis=AX.X, op=EQ.add)
        srt = sp.tile([128, BC], F32, tag="srt")
        A.activation(srt[0:k], ssq[0:k], AF.Sqrt)
        V.reciprocal(rnorm[0:k, ci, :, 0], srt[0:k])
        V.tensor_tensor(mn[0:k, ci], mt[0:k, ci],
                        rnorm[0:k, ci].broadcast_to([k, BC, CK]), op=EQ.mult)
        dq = sp.tile([128, BC, CK], F32, tag="dq")
        V.tensor_mul(dq[0:k], mt[0:k, ci], textb[0:k])
        dsum = sp.tile([128, BC], F32, tag="dsum")
        V.tensor_reduce(dsum[0:k], dq[0:k], axis=AX.X, op=EQ.add)
        V.tensor_mul(X[0:k, ci, 4:8], dsum[0:k], rnorm[0:k, ci, :, 0])

    # CLS excluded from z-stats
    V.memset(X[0:1, 0, 0:8], 0.0)
    # squares
    V.tensor_mul(X[:, :, 8:16], X[:, :, 0:8], X[:, :, 0:8])

    # ---- per-sample sums via ones-matmuls: [1, 16] ----
    st_ps = ps_misc.tile([1, 16], F32, tag="ps")
    for ci, (off, k) in enumerate(CHUNKS):
        T.matmul(st_ps[:, :], csb["c_onescol"][0:k, 0:1], X[0:k, ci, :],
                 start=(ci == 0), stop=(ci == 4))
    sums = pp.tile([1, 16], F32, tag="sums")
    A.copy(sums[:, :], st_ps[:, :])
    # var = (sumsq - sum^2/576)/575 ; ab = 0.5/(sqrt(var)+1e-6)
    musq = sp.tile([1, 8], F32, tag="musq")
    V.tensor_mul(musq[:], sums[:, 0:8], sums[:, 0:8])
    V.tensor_scalar_mul(musq[:], musq[:], -1.0 / (L - 1))
    var_ = sp.tile([1, 8], F32, tag="var_")
    V.tensor_add(var_[:], sums[:, 8:16], musq[:])
    stdv = sp.tile([1, 8], F32, tag="stdv")
    A.activation(stdv[:], var_[:], AF.Sqrt, scale=1.0 / (L - 2))
    V.tensor_scalar_add(stdv[:], stdv[:], 1e-6)
    inv = sp.tile([1, 8], F32, tag="inv")
    V.reciprocal(inv[:], stdv[:])
    ab_row = pp.tile([1, 8], F32, tag="ab_row")
    V.tensor_scalar_mul(ab_row[:], inv[:], 0.5)
    # partition-broadcast of ab: [128, 1, 8] PSUM
    abP = ps_misc.tile([128, 1, 8], F32, tag="ps")
    T.matmul(abP[:, 0, :], csb["c_ones1"][:, :], ab_row[:, :], start=True, stop=True)

    # ---- score_col = a*sd + b*cos ; CLS sentinel ----
    sc_t = sp.tile([128, 5, BC], F32, tag="sc_t")
    V.tensor_tensor(sc_t[:], X[:, :, 0:4],
                    abP[:, :, 0:4].broadcast_to([128, 5, 4]), op=EQ.mult)
    sc_u = sp.tile([128, 5, BC], F32, tag="sc_u")
    V.tensor_tensor(sc_u[:], X[:, :, 4:8],
                    abP[:, :, 4:8].broadcast_to([128, 5, 4]), op=EQ.mult)
    score_col = pp.tile([128, 5, BC], F32, tag="score_col")
    V.tensor_add(score_col[:], sc_t[:], sc_u[:])
    V.memset(score_col[0:1, 0, :], 1.0e30)

    if stage <= 2:
        return dump(2)

    # ---- score_row [BC, 640] via 5 transposes ----
    score_row = pp.tile([BC, LPAD], F32, tag="score_row")
    for ci, (off, k) in enumerate(CHUNKS):
        srp = ps_misc.tile([BC, 128], F32, tag="ps")
        T.transpose(srp[:, 0:k], score_col[0:k, ci, :], csb["c_iden"][0:k, 0:k])
        A.copy(score_row[:, off:off + k], srp[:, 0:k])

    # ---- mnT (overlaps rank below; PE/ACT while V/G rank) ----
    mnT = []
    for s in range(BC):
        t = pp.tile([CK, LPAD], F32, tag=f"mnT{s}")
        for ci, (off, k) in enumerate(CHUNKS):
            tps = ps_misc.tile([CK, 128], F32, tag="ps")
            T.transpose(tps[:, 0:k], mn[0:k, ci, s, :], csb["c_iden"][0:k, 0:k])
            A.copy(t[:, off:off + k], tps[:, 0:k])
        mnT.append(t)

    # ---- rank: per-sample broadcast (PSUM) + compare-accum (V/G split) ----
    rank = pp.tile([128, 5, BC], F32, tag="rank")
    G.memset(rank[:].rearrange("p c s -> p (c s)"), 1.0e9)
    # samples 0,1: Vector fused compare+accum; samples 2,3: GpSimd transposed
    # compares (gT[j,i] = s_j > s_i) + PE ones-matmul partition reduce +
    # tiny transposes back to column form. bcs broadcast via SBUF->SBUF DMA
    # with partition-stride-0 source (exact fp32, no PE cost).
    for s in [0, 2, 1, 3]:
        bc_ps = ps_bcs.tile([128, LPAD], F32, tag="bcs")
        T.matmul(bc_ps[:, 0:512], csb["c_selbc"][:, s * 128:(s + 1) * 128],
                 score_row[:, 0:512], start=True, stop=True)
        T.matmul(bc_ps[:, 512:LPAD], csb["c_selbc"][:, s * 128:(s + 1) * 128],
                 score_row[:, 512:LPAD], start=True, stop=True)
        if s < 2:
            for ci, (off, k) in enumerate(CHUNKS):
                g = sp.tile([128, LPAD], F32, tag="g")
                V.tensor_scalar(g[0:k, 0:L], bc_ps[0:k, 0:L],
                                score_col[0:k, ci, s:s + 1], 0.0,
                                op0=EQ.is_gt, op1=EQ.add,
                                accum_out=rank[0:k, ci, s:s + 1])
        else:
            bcs_sb = sp.tile([128, LPAD], F32, tag="bcs_sb")
            A.copy(bcs_sb[:, 0:L], bc_ps[:, 0:L])
            rr = ps_rr.tile([1, LPAD], F32, tag="rr")
            for cj, (joff, kj) in enumerate(CHUNKS):
                gT = sp.tile([128, LPAD], BF16, tag="gT")
                G.tensor_scalar(gT[0:kj, 0:L], bcs_sb[0:kj, 0:L],
                                score_col[0:kj, cj, s:s + 1], None,
                                op0=EQ.is_lt)
                T.matmul(rr[0:1, 0:512], csb["c_onescolb"][0:kj, 0:1],
                         gT[0:kj, 0:512], start=(cj == 0), stop=(cj == 4))
                T.matmul(rr[0:1, 512:LPAD], csb["c_onescolb"][0:kj, 0:1],
                         gT[0:kj, 512:LPAD], start=(cj == 0), stop=(cj == 4))
            rr_sb = sp.tile([1, LPAD], F32, tag="rr_sb")
            A.copy(rr_sb[:, 0:L], rr[0:1, 0:L])
            for ci, (off, k) in enumerate(CHUNKS):
                rt = ps_misc.tile([128, 1], F32, tag="ps")
                T.transpose(rt[0:k, :], rr_sb[0:1, off:off + k],
                            csb["c_iden"][0:1, 0:1])
                A.copy(rank[0:k, ci, s:s + 1], rt[0:k, :])

    if stage <= 3:
        return dump(3)

    # ---- msk (f32 + bf16), notm, cums, pn ----
    msk_f = pp.tile([128, 5, BC, 1], F32, tag="msk_f")
    V.tensor_scalar(msk_f[:].rearrange("p c s o -> p (c s o)"),
                    rank[:].rearrange("p c s -> p (c s)"),
                    float(NSEL), None, op0=EQ.is_lt)
    msk_b = pp.tile([128, 5, BC, 1], BF16, tag="msk_b")
    G.tensor_scalar(msk_b[:].rearrange("p c s o -> p (c s o)"),
                    rank[:].rearrange("p c s -> p (c s)"),
                    float(NSEL), None, op0=EQ.is_lt)
    notm = pp.tile([128, 5, BC, 1], F32, tag="notm")
    G.tensor_scalar(notm[:].rearrange("p c s o -> p (c s o)"),
                    msk_f[:].rearrange("p c s o -> p (c s o)"),
                    0.5, None, op0=EQ.is_lt)
    cums = pp.tile([128, 5, BC, 1], F32, tag="cums")
    G.memset(cums[:].rearrange("p c s o -> p (c s o)"), 0.0)
    for cm in range(5):
        kcm = CHUNKS[cm][1]
        cps = ps_misc.tile([128, BC], F32, tag="ps")
        for ck in range(cm + 1):
            lhs = csb["c_utb"] if ck == cm else csb["c_onesb"]
            kk = CHUNKS[ck][1]
            T.matmul(cps[0:kcm, :], lhs[0:kk, 0:kcm], msk_b[0:kk, ck, :, 0],
                     start=(ck == 0), stop=(ck == cm))
        A.copy(cums[0:kcm, cm, :, 0], cps[0:kcm, :])
    pn = pp.tile([128, 5, BC, 1], F32, tag="pn")
    V.tensor_tensor(pn[:, :, :, 0], cums[:, :, :, 0],
                    csb["c_iotaI"][:].broadcast_to([128, 5, BC]), op=EQ.subtract)

    if stage <= 4:
        return dump(4)

    # ---- itgt, ismrg ----
    itgt = pp.tile([128, 5, BC, CTX], F32, tag="itgt")
    ismrg = pp.tile([128, 5, BC, 1], F32, tag="ismrg")
    G.memset(ismrg[:].rearrange("p c s o -> p (c s o)"), 0.0)
    for ci, (off, k) in enumerate(CHUNKS):
        V.tensor_tensor(itgt[0:k, ci], csb["c_iota52"][0:k].broadcast_to([k, BC, CTX]),
                        pn[0:k, ci].broadcast_to([k, BC, CTX]), op=EQ.is_equal)
        V.tensor_tensor(itgt[0:k, ci], itgt[0:k, ci],
                        notm[0:k, ci].broadcast_to([k, BC, CTX]), op=EQ.mult)
        tany = sp.tile([128, BC], F32, tag="tany")
        V.tensor_reduce(tany[0:k], itgt[0:k, ci], axis=AX.X, op=EQ.add)
        omt = sp.tile([128, BC], F32, tag="omt")
        G.tensor_scalar(omt[0:k], tany[0:k], -1.0, 1.0, op0=EQ.mult, op1=EQ.add)
        G.tensor_mul(ismrg[0:k, ci, :, 0], notm[0:k, ci, :, 0], omt[0:k])

    # ---- Tn: [CK, BC, CTX] (batched PSUM) ----
    tn_ps = ps_misc.tile([CK, BC, CTX], F32, tag="ps")
    for s in range(BC):
        for ci, (off, k) in enumerate(CHUNKS):
            T.matmul(tn_ps[:, s, :], mn[0:k, ci, s, :], itgt[0:k, ci, s, :],
                     start=(ci == 0), stop=(ci == 4))
    tn_sb = pp.tile([CK, BC, CTX], F32, tag="tn_sb")
    A.copy(tn_sb[:].rearrange("p s c -> p (s c)"),
           tn_ps[:].rearrange("p s c -> p (s c)"))

    if stage <= 5:
        return dump(5)

    # ---- sim (batched PSUM per chunk), rmx, eqm ----
    eqm = pp.tile([128, 5, BC, CTX], F32, tag="eqm")
    for ci, (off, k) in enumerate(CHUNKS):
        sim_ps = ps_misc.tile([128, BC, CTX], F32, tag="ps")
        for s in range(BC):
            T.matmul(sim_ps[0:k, s, :], mnT[s][:, off:off + k], tn_sb[:, s, :],
                     start=True, stop=True)
        rmx = sp.tile([128, BC, 1], F32, tag="rmx")
        V.tensor_reduce(rmx[0:k, :, 0], sim_ps[0:k], axis=AX.X, op=EQ.max)
        V.tensor_tensor(eqm[0:k, ci], sim_ps[0:k],
                        rmx[0:k].broadcast_to([k, BC, CTX]), op=EQ.is_ge)
        V.tensor_tensor(eqm[0:k, ci], eqm[0:k, ci],
                        ismrg[0:k, ci].broadcast_to([k, BC, CTX]), op=EQ.mult)

    if stage <= 6:
        return dump(6)

    # ---- counts -> 1/cnt, partition-broadcast ----
    cnt_ps = ps_misc.tile([BC * CTX, 1], F32, tag="ps")
    for ci, (off, k) in enumerate(CHUNKS):
        T.matmul(cnt_ps[:, :], eqm[0:k, ci].rearrange("p s c -> p (s c)"),
                 csb["c_onescol"][0:k, :], start=(ci == 0), stop=(ci == 4))
    cmax = sp.tile([BC * CTX, 1], F32, tag="cmax")
    V.tensor_scalar_max(cmax[:], cnt_ps[:, :], 1.0)
    crec = sp.tile([BC * CTX, 1], F32, tag="crec")
    V.reciprocal(crec[:], cmax[:])
    crT_ps = ps_misc.tile([1, BC * CTX], F32, tag="ps")
    T.transpose(crT_ps[:, :], crec[:, :], csb["c_iden"][0:BC * CTX, 0:BC * CTX])
    crec_row = sp.tile([1, BC * CTX], F32, tag="crec_row")
    A.copy(crec_row[:, :], crT_ps[:, :])
    crb_ps = ps_misc.tile([128, BC, CTX], F32, tag="ps")
    T.matmul(crb_ps[:].rearrange("p s c -> p (s c)"), csb["c_ones1"][:, :],
             crec_row[:, :], start=True, stop=True)
    crb = pp.tile([128, BC, CTX], F32, tag="crb")
    A.copy(crb[:].rearrange("p s c -> p (s c)"),
           crb_ps[:].rearrange("p s c -> p (s c)"))

    if stage <= 7:
        return dump(7)

    # ---- C build (bf16): rows 0..54 one-hots, rows 55.. itgt + eqm/cnt ----
    cts = pp.tile([128, 5, BC, 80], BF16, tag="cts")
    for ci, (off, k) in enumerate(CHUNKS):
        dom = sp.tile([128, BC, NSEL], F32, tag="dom")
        V.tensor_tensor(dom[0:k], csb["c_iota55"][0:k].broadcast_to([k, BC, NSEL]),
                        cums[0:k, ci].broadcast_to([k, BC, NSEL]), op=EQ.is_equal)
        V.tensor_tensor(cts[0:k, ci, :, 0:NSEL], dom[0:k],
                        msk_f[0:k, ci].broadcast_to([k, BC, NSEL]), op=EQ.mult)
        wct = sp.tile([128, BC, CTX], F32, tag="wct")
        G.tensor_mul(wct[0:k], eqm[0:k, ci], crb[0:k])
        G.tensor_add(cts[0:k, ci, :, NSEL:OUT_T], wct[0:k], itgt[0:k, ci])

    if stage <= 8:
        return dump(8)

    # ---- big matmuls (bf16) + copy + out DMA ----
    for s in range(BC):
        for n2 in range(2):
            po = ps_out.tile([OUT_T, 512], F32, tag="po")
            for ci, (off, k) in enumerate(CHUNKS):
                T.matmul(po[:, :], cts[0:k, ci, s, 0:OUT_T],
                         hid[s][ci][0:k, n2 * 512:(n2 + 1) * 512],
                         start=(ci == 0), stop=(ci == 4))
            ob = sp.tile([OUT_T, 512], F32, tag="ob", bufs=3)
            if (s * 2 + n2) % 2 == 0:
                A.copy(ob[:, :], po[:, :])
            else:
                V.tensor_scalar_add(ob[:, :], po[:, :], 0.0)
            DMA2.dma_start(out_d[s, :, n2 * 512:(n2 + 1) * 512], ob[:, :])


_NC = None


def _get_nc():
    global _NC
    if _NC is None:
        _NC = build_nc()
    return _NC


def shard_inputs(attn_weights, hidden_states, metric, text_emb):
    """Host-side shard: slice CLS attention row, transpose to token-major
    column layout, cast hidden to bf16, split batch across cores."""
    B = attn_weights.shape[0]
    per = B // N_CORES
    attn_row = np.ascontiguousarray(attn_weights[:, :, 0, :], dtype=np.float32)
    h_b = np.asarray(hidden_states, np.float32).astype(NPBF16)
    met = np.asarray(metric, np.float32)
    consts = _consts()
    in_maps = []
    for c in range(N_CORES):
        sl = slice(c * per, (c + 1) * per)
        # attnT: [4,16,577] -> [577,4,16] -> pad 640 -> [128, 5, 64]
        at = attn_row[sl].transpose(2, 0, 1)                   # [577, 4, 16]
        atp = np.zeros((LPAD, per, NH), np.float32)
        atp[:L] = at
        atT = np.ascontiguousarray(
            atp.reshape(5, 128, per * NH).transpose(1, 0, 2))  # [128, 5, 64]
        # metricT: [4,577,64] -> [577,4,64] -> pad 640 -> [128, 5, 4, 64]
        mtc = met[sl].transpose(1, 0, 2)                       # [577, 4, 64]
        mtp = np.zeros((LPAD, per, CK), np.float32)
        mtp[:L] = mtc
        mtT = np.ascontiguousarray(
            mtp.reshape(5, 128, per, CK).transpose(1, 0, 2, 3))
        m = {
            "attnT": atT,
            "metricT": mtT,
            "text": np.ascontiguousarray(text_emb[sl]).astype(np.float32),
            "hidb": np.ascontiguousarray(h_b[sl]),
        }
        m.update(consts)
        in_maps.append(m)
    return in_maps


def kernel(attn_weights, hidden_states, metric, text_emb):
    nc = _get_nc()
    in_maps = shard_inputs(attn_weights, hidden_states, metric, text_emb)
    res = run_bass_kernel_spmd(nc, in_maps, core_ids=list(range(N_CORES)))
    out = np.concatenate([r["out"] for r in res.results], axis=0)
    return out.astype(np.float32)


# revision 13
# speedup vs baseline: 2.0135x; 2.0135x over previous
"""VisionZip text-aware token-selection kernel for Trainium2 (Bass/Tile), v2.

Contract: kernel(**inputs) takes FULL inputs (B=32) and returns the FULL
output [32, 65, 1024]. Pure data-parallel over 8 NeuronCores (4 samples each).

v2 changes vs baseline:
  - hidden passed as single bf16 copy (fp16 runs at 2 cycles/row on the PE;
    bf16 runs at 1) -> big matmul phase ~4x faster, hidden DMA halved.
  - 1/cnt folded into the C matrix (C_ctx = itgt + eqm/cnt), so the PSUM
    result is final: no per-output recip multiply, fewer count matmuls.
  - affine score trick: rank order of 0.5*z(Sd)+0.5*z(cos) equals rank order
    of a*Sd + b*cos with a=0.5/(std_sd+eps), b=0.5/(std_cos+eps) (per-sample
    constants cancel) -> no mean-subtraction passes over [*,576].
  - column-major score pipeline: attn CLS-row and metric are host-transposed
    to [token(128p), chunk, ...] so Sd is one X-reduce and all selection
    tensors stay in the [128, 5, BC] layout; sums/sumsqs via ones-matmuls.
  - rank compare+accum ops read the broadcast scores directly from PSUM and
    are split across Vector and GpSimd.
  - sim/Tn matmuls batched into single PSUM tiles per chunk; rmx/eqm read
    PSUM directly (no sim copies).
  - hidden DMAs on the scalar-engine HWDGE ring, small inputs on the sync
    ring (parallel input streams); outputs also on the scalar ring.
"""
import numpy as np

import sys
if '/opt/trn_rl_repo' not in sys.path:
    sys.path.insert(0, '/opt/trn_rl_repo')

import concourse.bacc as bacc
import concourse.tile as tile
from concourse import mybir
from concourse.bass_utils import run_bass_kernel_spmd

F32 = mybir.dt.float32
F32R = mybir.dt.float32r
BF16 = mybir.dt.bfloat16
NPBF16 = mybir.dt.np(mybir.dt.bfloat16)
N_CORES = 8
BC = 4                      # samples per core
L = 577                     # tokens (incl CLS)
LPAD = 640
D = 1024
CK = 64
NH = 16
DOM = 54
NSEL = DOM + 1              # + CLS
CTX = 10
STEP = 52                   # (577-1-54) // 10
OUT_T = NSEL + CTX          # 65 output tokens
CHUNKS = [(0, 128), (128, 128), (256, 128), (384, 128), (512, 65)]
EQ = mybir.AluOpType
AF = mybir.ActivationFunctionType
AX = mybir.AxisListType


def _consts():
    c = {}
    c["c_iden"] = np.eye(128, dtype=np.float32)
    ut = (np.arange(128)[:, None] <= np.arange(128)[None, :]).astype(NPBF16)
    c["c_utb"] = ut
    c["c_onesb"] = np.ones((128, 128), NPBF16)
    c["c_onescol"] = np.ones((128, 1), np.float32)
    c["c_onescolb"] = np.ones((128, 1), NPBF16)
    c["c_ones1"] = np.ones((1, 128), np.float32)
    selbc = np.zeros((BC, BC * 128), np.float32)
    for s in range(BC):
        selbc[s, s * 128:(s + 1) * 128] = 1.0
    c["c_selbc"] = selbc
    c["c_iota55"] = (np.arange(NSEL) + 1.0).astype(np.float32).reshape(1, 1, NSEL) \
        .repeat(128, 0).copy()
    c["c_iota52"] = (-STEP * np.arange(CTX, dtype=np.float32)).reshape(1, 1, CTX) \
        .repeat(128, 0).copy()
    ii = np.zeros((128, 5, 1), np.float32)
    for ci, (off, _) in enumerate(CHUNKS):
        ii[:, ci, 0] = off + np.arange(128)
    c["c_iotaI"] = ii
    return c


_CONST_DTYPES = {"c_utb": BF16, "c_onesb": BF16, "c_onescolb": BF16}


def build_nc(stage=99):
    nc = bacc.Bacc("TRN2", target_bir_lowering=False, debug=False)

    attnT_d = nc.declare_dram_parameter("attnT", [128, 5, BC * NH], F32, isOutput=False)
    metricT_d = nc.declare_dram_parameter("metricT", [128, 5, BC, CK], F32, isOutput=False)
    text_d = nc.declare_dram_parameter("text", [BC, CK], F32, isOutput=False)
    hid_d = nc.declare_dram_parameter("hidb", [BC, L, D], BF16, isOutput=False)
    cshapes = {k: v.shape for k, v in _consts().items()}
    cdram = {k: nc.declare_dram_parameter(k, list(sh), _CONST_DTYPES.get(k, F32),
                                          isOutput=False)
             for k, sh in cshapes.items()}
    out_d = nc.declare_dram_parameter("out", [BC, OUT_T, D], F32, isOutput=True)

    with tile.TileContext(nc) as tc:
        with (
            tc.tile_pool(name="persist", bufs=1) as pp,
            tc.tile_pool(name="hidpool", bufs=1) as hp,
            tc.tile_pool(name="scratch", bufs=2) as sp,
            tc.tile_pool(name="ps_misc", bufs=2, space="PSUM") as ps_misc,
            tc.tile_pool(name="ps_bcs", bufs=2, space="PSUM") as ps_bcs,
            tc.tile_pool(name="ps_out", bufs=2, space="PSUM") as ps_out,
        ):
            pools = (pp, hp, sp, ps_misc, ps_bcs, ps_out)
            _body(nc, stage, pools, attnT_d, metricT_d, text_d, hid_d,
                  cdram, cshapes, out_d)
    nc.compile()
    return nc


def _body(nc, stage, pools, attnT_d, metricT_d, text_d, hid_d,
          cdram, cshapes, out_d):
    pp, hp, sp, ps_misc, ps_bcs, ps_out = pools
    V = nc.vector
    A = nc.scalar
    G = nc.gpsimd
    T = nc.tensor
    DMA = nc.sync          # small inputs: sync-engine HWDGE ring
    DMA2 = nc.scalar       # hidden + outputs: scalar-engine HWDGE ring

    def dump(n):
        d = sp.tile([BC, 512], F32, tag="dump")
        V.memset(d[:], float(n))
        DMA.dma_start(out_d[:, 0, 0:512], d[:])

    # ---- input DMAs ----
    attnT = pp.tile([128, 5, BC * NH], F32, tag="attnT")
    DMA.dma_start(attnT[:], attnT_d[:])
    text_sb = pp.tile([BC, CK], F32, tag="text_sb")
    DMA.dma_start(text_sb[:], text_d[:])
    mt = pp.tile([128, 5, BC, CK], F32, tag="mt")
    DMA.dma_start(mt[:], metricT_d[:])
    csb = {}
    for k, sh in cshapes.items():
        t = pp.tile(list(sh), _CONST_DTYPES.get(k, F32), tag=k)
        DMA.dma_start(t[:], cdram[k][:])
        csb[k] = t
    # hidden (big): sync ring, queued after the small inputs
    hid = []
    for s in range(BC):
        row = []
        for ci, (off, k) in enumerate(CHUNKS):
            th = hp.tile([128, D], BF16, tag=f"h{s}_{ci}")
            DMA.dma_start(th[0:k, :], hid_d[s, off:off + k, :])
            row.append(th)
        hid.append(row)
    # prefetch the ACT Sqrt table during DMA wait
    dmt = sp.tile([1, 1], F32, tag="dmt")
    V.memset(dmt[:], 1.0)
    dmt2 = sp.tile([1, 1], F32, tag="dmt2")
    A.activation(dmt2[:], dmt[:], AF.Sqrt)

    if stage <= 1:
        return dump(1)

    # ---- text_n and its partition-broadcast ----
    tsc = sp.tile([BC, CK], F32, tag="tsc")
    V.tensor_mul(tsc[:], text_sb[:], text_sb[:])
    tss = pp.tile([BC, 1], F32, tag="tss")
    V.tensor_reduce(tss[:], tsc[:], axis=AX.X, op=EQ.add)
    tst = pp.tile([BC, 1], F32, tag="tst")
    A.activation(tst[:], tss[:], AF.Sqrt)
    trc = pp.tile([BC, 1], F32, tag="trc")
    V.reciprocal(trc[:], tst[:])
    textn = pp.tile([BC, CK], F32, tag="textn")
    V.tensor_scalar_mul(textn[:], text_sb[:], trc[:])
    tb_ps = ps_misc.tile([128, BC * CK], F32, tag="ps")
    for s in range(BC):
        T.matmul(tb_ps[:, s * CK:(s + 1) * CK],
                 csb["c_selbc"][:, s * 128:(s + 1) * 128],
                 textn[:, :], start=True, stop=True)
    textb = pp.tile([128, BC, CK], F32, tag="textb")
    A.copy(textb[:].rearrange("p s c -> p (s c)"), tb_ps[:, :])

    # ---- X tile: [128, 5, (sd s0..3 | cos s0..3 | sd^2 | cos^2)] ----
    X = pp.tile([128, 5, 16], F32, tag="X")
    # Sd: sum CLS-attention over 16 heads -> X[:, :, 0:4]
    V.tensor_reduce(X[:, :, 0:4], attnT[:].rearrange("p c (s h) -> p c s h", h=NH),
                    axis=AX.X, op=EQ.add)

    # ---- metric norms, mn, cos ----
    mn = pp.tile([128, 5, BC, CK], F32, tag="mn")
    rnorm = pp.tile([128, 5, BC, 1], F32, tag="rnorm")
    for ci, (off, k) in enumerate(CHUNKS):
        sq = sp.tile([128, BC, CK], F32, tag="sq")
        V.tensor_mul(sq[0:k], mt[0:k, ci], mt[0:k, ci])
        ssq = sp.tile([128, BC], F32, tag="ssq")
        V.tensor_reduce(ssq[0:k], sq[0:k], axis=AX.X, op=EQ.add)
        srt = sp.tile([128, BC], F32, tag="srt")
        A.activation(srt[0:k], ssq[0:k], AF.Sqrt)
        V.reciprocal(rnorm[0:k, ci, :, 0], srt[0:k])
        V.tensor_tensor(mn[0:k, ci], mt[0:k, ci],
                        rnorm[0:k, ci].broadcast_to([k, BC, CK]), op=EQ.mult)
        dq = sp.tile([128, BC, CK], F32, tag="dq")
        V.tensor_mul(dq[0:k], mt[0:k, ci], textb[0:k])
        dsum = sp.tile([128, BC], F32, tag="dsum")
        V.tensor_reduce(dsum[0:k], dq[0:k], axis=AX.X, op=EQ.add)
        V.tensor_mul(X[0:k, ci, 4:8], dsum[0:k], rnorm[0:k, ci, :, 0])

    # CLS excluded from z-stats
    V.memset(X[0:1, 0, 0:8], 0.0)
    # squares
    V.tensor_mul(X[:, :, 8:16], X[:, :, 0:8], X[:, :, 0:8])

    # ---- per-sample sums via ones-matmuls: [1, 16] ----
    st_ps = ps_misc.tile([1, 16], F32, tag="ps")
    for ci, (off, k) in enumerate(CHUNKS):
        T.matmul(st_ps[:, :], csb["c_onescol"][0:k, 0:1], X[0:k, ci, :],
                 start=(ci == 0), stop=(ci == 4))
    sums = pp.tile([1, 16], F32, tag="sums")
    A.copy(sums[:, :], st_ps[:, :])
    # var = (sumsq - sum^2/576)/575 ; ab = 0.5/(sqrt(var)+1e-6)
    musq = sp.tile([1, 8], F32, tag="musq")
    V.tensor_mul(musq[:], sums[:, 0:8], sums[:, 0:8])
    V.tensor_scalar_mul(musq[:], musq[:], -1.0 / (L - 1))
    var_ = sp.tile([1, 8], F32, tag="var_")
    V.tensor_add(var_[:], sums[:, 8:16], musq[:])
    stdv = sp.tile([1, 8], F32, tag="stdv")
    A.activation(stdv[:], var_[:], AF.Sqrt, scale=1.0 / (L - 2))
    V.tensor_scalar_add(stdv[:], stdv[:], 1e-6)
    inv = sp.tile([1, 8], F32, tag="inv")
    V.reciprocal(inv[:], stdv[:])
    ab_row = pp.tile([1, 8], F32, tag="ab_row")
    V.tensor_scalar_mul(ab_row[:], inv[:], 0.5)
    # partition-broadcast of ab: [128, 1, 8] PSUM
    abP = ps_misc.tile([128, 1, 8], F32, tag="ps")
    T.matmul(abP[:, 0, :], csb["c_ones1"][:, :], ab_row[:, :], start=True, stop=True)

    # ---- score_col = a*sd + b*cos ; CLS sentinel ----
    sc_t = sp.tile([128, 5, BC], F32, tag="sc_t")
    V.tensor_tensor(sc_t[:], X[:, :, 0:4],
                    abP[:, :, 0:4].broadcast_to([128, 5, 4]), op=EQ.mult)
    sc_u = sp.tile([128, 5, BC], F32, tag="sc_u")
    V.tensor_tensor(sc_u[:], X[:, :, 4:8],
                    abP[:, :, 4:8].broadcast_to([128, 5, 4]), op=EQ.mult)
    score_col = pp.tile([128, 5, BC], F32, tag="score_col")
    V.tensor_add(score_col[:], sc_t[:], sc_u[:])
    V.memset(score_col[0:1, 0, :], 1.0e30)

    if stage <= 2:
        return dump(2)

    # ---- score_row [BC, 640] via 5 transposes ----
    score_row = pp.tile([BC, LPAD], F32, tag="score_row")
    for ci, (off, k) in enumerate(CHUNKS):
        srp = ps_misc.tile([BC, 128], F32, tag="ps")
        T.transpose(srp[:, 0:k], score_col[0:k, ci, :], csb["c_iden"][0:k, 0:k])
        A.copy(score_row[:, off:off + k], srp[:, 0:k])

    # ---- mnT (overlaps rank below; PE/ACT while V/G rank) ----
    mnT = []
    for s in range(BC):
        t = pp.tile([CK, LPAD], F32, tag=f"mnT{s}")
        for ci, (off, k) in enumerate(CHUNKS):
            tps = ps_misc.tile([CK, 128], F32, tag="ps")
            T.transpose(tps[:, 0:k], mn[0:k, ci, s, :], csb["c_iden"][0:k, 0:k])
            A.copy(t[:, off:off + k], tps[:, 0:k])
        mnT.append(t)

    # ---- rank: per-sample broadcast (PSUM) + compare-accum (V/G split) ----
    rank = pp.tile([128, 5, BC], F32, tag="rank")
    G.memset(rank[:].rearrange("p c s -> p (c s)"), 1.0e9)
    for s in range(BC):
        bc_ps = ps_bcs.tile([128, LPAD], F32, tag="bcs")
        T.matmul(bc_ps[:, 0:512], csb["c_selbc"][:, s * 128:(s + 1) * 128],
                 score_row[:, 0:512], start=True, stop=True)
        T.matmul(bc_ps[:, 512:LPAD], csb["c_selbc"][:, s * 128:(s + 1) * 128],
                 score_row[:, 512:LPAD], start=True, stop=True)
        for ci, (off, k) in enumerate(CHUNKS):
            g = sp.tile([128, LPAD], F32, tag="g")
            V.tensor_scalar(g[0:k, 0:L], bc_ps[0:k, 0:L],
                            score_col[0:k, ci, s:s + 1], 0.0,
                            op0=EQ.is_gt, op1=EQ.add,
                            accum_out=rank[0:k, ci, s:s + 1])

    if stage <= 3:
        return dump(3)

    # ---- msk (f32 + bf16), notm, cums, pn ----
    msk_f = pp.tile([128, 5, BC, 1], F32, tag="msk_f")
    V.tensor_scalar(msk_f[:].rearrange("p c s o -> p (c s o)"),
                    rank[:].rearrange("p c s -> p (c s)"),
                    float(NSEL), None, op0=EQ.is_lt)
    msk_b = pp.tile([128, 5, BC, 1], BF16, tag="msk_b")
    G.tensor_scalar(msk_b[:].rearrange("p c s o -> p (c s o)"),
                    rank[:].rearrange("p c s -> p (c s)"),
                    float(NSEL), None, op0=EQ.is_lt)
    notm = pp.tile([128, 5, BC, 1], F32, tag="notm")
    G.tensor_scalar(notm[:].rearrange("p c s o -> p (c s o)"),
                    msk_f[:].rearrange("p c s o -> p (c s o)"),
                    0.5, None, op0=EQ.is_lt)
    cums = pp.tile([128, 5, BC, 1], F32, tag="cums")
    G.memset(cums[:].rearrange("p c s o -> p (c s o)"), 0.0)
    for cm in range(5):
        kcm = CHUNKS[cm][1]
        cps = ps_misc.tile([128, BC], F32, tag="ps")
        for ck in range(cm + 1):
            lhs = csb["c_utb"] if ck == cm else csb["c_onesb"]
            kk = CHUNKS[ck][1]
            T.matmul(cps[0:kcm, :], lhs[0:kk, 0:kcm], msk_b[0:kk, ck, :, 0],
                     start=(ck == 0), stop=(ck == cm))
        A.copy(cums[0:kcm, cm, :, 0], cps[0:kcm, :])
    pn = pp.tile([128, 5, BC, 1], F32, tag="pn")
    V.tensor_tensor(pn[:, :, :, 0], cums[:, :, :, 0],
                    csb["c_iotaI"][:].broadcast_to([128, 5, BC]), op=EQ.subtract)

    if stage <= 4:
        return dump(4)

    # ---- itgt, ismrg ----
    itgt = pp.tile([128, 5, BC, CTX], F32, tag="itgt")
    ismrg = pp.tile([128, 5, BC, 1], F32, tag="ismrg")
    G.memset(ismrg[:].rearrange("p c s o -> p (c s o)"), 0.0)
    for ci, (off, k) in enumerate(CHUNKS):
        V.tensor_tensor(itgt[0:k, ci], csb["c_iota52"][0:k].broadcast_to([k, BC, CTX]),
                        pn[0:k, ci].broadcast_to([k, BC, CTX]), op=EQ.is_equal)
        V.tensor_tensor(itgt[0:k, ci], itgt[0:k, ci],
                        notm[0:k, ci].broadcast_to([k, BC, CTX]), op=EQ.mult)
        tany = sp.tile([128, BC], F32, tag="tany")
        V.tensor_reduce(tany[0:k], itgt[0:k, ci], axis=AX.X, op=EQ.add)
        omt = sp.tile([128, BC], F32, tag="omt")
        G.tensor_scalar(omt[0:k], tany[0:k], -1.0, 1.0, op0=EQ.mult, op1=EQ.add)
        G.tensor_mul(ismrg[0:k, ci, :, 0], notm[0:k, ci, :, 0], omt[0:k])

    # ---- Tn: [CK, BC, CTX] (batched PSUM) ----
    tn_ps = ps_misc.tile([CK, BC, CTX], F32, tag="ps")
    for s in range(BC):
        for ci, (off, k) in enumerate(CHUNKS):
            T.matmul(tn_ps[:, s, :], mn[0:k, ci, s, :], itgt[0:k, ci, s, :],
                     start=(ci == 0), stop=(ci == 4))
    tn_sb = pp.tile([CK, BC, CTX], F32, tag="tn_sb")
    A.copy(tn_sb[:].rearrange("p s c -> p (s c)"),
           tn_ps[:].rearrange("p s c -> p (s c)"))

    if stage <= 5:
        return dump(5)

    # ---- sim (batched PSUM per chunk), rmx, eqm ----
    eqm = pp.tile([128, 5, BC, CTX], F32, tag="eqm")
    for ci, (off, k) in enumerate(CHUNKS):
        sim_ps = ps_misc.tile([128, BC, CTX], F32, tag="ps")
        for s in range(BC):
            T.matmul(sim_ps[0:k, s, :], mnT[s][:, off:off + k], tn_sb[:, s, :],
                     start=True, stop=True)
        rmx = sp.tile([128, BC, 1], F32, tag="rmx")
        V.tensor_reduce(rmx[0:k, :, 0], sim_ps[0:k], axis=AX.X, op=EQ.max)
        V.tensor_tensor(eqm[0:k, ci], sim_ps[0:k],
                        rmx[0:k].broadcast_to([k, BC, CTX]), op=EQ.is_ge)
        V.tensor_tensor(eqm[0:k, ci], eqm[0:k, ci],
                        ismrg[0:k, ci].broadcast_to([k, BC, CTX]), op=EQ.mult)

    if stage <= 6:
        return dump(6)

    # ---- counts -> 1/cnt, partition-broadcast ----
    cnt_ps = ps_misc.tile([BC * CTX, 1], F32, tag="ps")
    for ci, (off, k) in enumerate(CHUNKS):
        T.matmul(cnt_ps[:, :], eqm[0:k, ci].rearrange("p s c -> p (s c)"),
                 csb["c_onescol"][0:k, :], start=(ci == 0), stop=(ci == 4))
    cmax = sp.tile([BC * CTX, 1], F32, tag="cmax")
    V.tensor_scalar_max(cmax[:], cnt_ps[:, :], 1.0)
    crec = sp.tile([BC * CTX, 1], F32, tag="crec")
    V.reciprocal(crec[:], cmax[:])
    crT_ps = ps_misc.tile([1, BC * CTX], F32, tag="ps")
    T.transpose(crT_ps[:, :], crec[:, :], csb["c_iden"][0:BC * CTX, 0:BC * CTX])
    crec_row = sp.tile([1, BC * CTX], F32, tag="crec_row")
    A.copy(crec_row[:, :], crT_ps[:, :])
    crb_ps = ps_misc.tile([128, BC, CTX], F32, tag="ps")
    T.matmul(crb_ps[:].rearrange("p s c -> p (s c)"), csb["c_ones1"][:, :],
             crec_row[:, :], start=True, stop=True)
    crb = pp.tile([128, BC, CTX], F32, tag="crb")
    A.copy(crb[:].rearrange("p s c -> p (s c)"),
           crb_ps[:].rearrange("p s c -> p (s c)"))

    if stage <= 7:
        return dump(7)

    # ---- C build (bf16): rows 0..54 one-hots, rows 55.. itgt + eqm/cnt ----
    cts = pp.tile([128, 5, BC, 80], BF16, tag="cts")
    for ci, (off, k) in enumerate(CHUNKS):
        dom = sp.tile([128, BC, NSEL], F32, tag="dom")
        V.tensor_tensor(dom[0:k], csb["c_iota55"][0:k].broadcast_to([k, BC, NSEL]),
                        cums[0:k, ci].broadcast_to([k, BC, NSEL]), op=EQ.is_equal)
        V.tensor_tensor(cts[0:k, ci, :, 0:NSEL], dom[0:k],
                        msk_f[0:k, ci].broadcast_to([k, BC, NSEL]), op=EQ.mult)
        wct = sp.tile([128, BC, CTX], F32, tag="wct")
        V.tensor_mul(wct[0:k], eqm[0:k, ci], crb[0:k])
        V.tensor_add(cts[0:k, ci, :, NSEL:OUT_T], wct[0:k], itgt[0:k, ci])

    if stage <= 8:
        return dump(8)

    # ---- big matmuls (bf16) + copy + out DMA ----
    for s in range(BC):
        for n2 in range(2):
            po = ps_out.tile([OUT_T, 512], F32, tag="po")
            for ci, (off, k) in enumerate(CHUNKS):
                T.matmul(po[:, :], cts[0:k, ci, s, 0:OUT_T],
                         hid[s][ci][0:k, n2 * 512:(n2 + 1) * 512],
                         start=(ci == 0), stop=(ci == 4))
            ob = sp.tile([OUT_T, 512], F32, tag="ob", bufs=3)
            if (s * 2 + n2) % 2 == 0:
                A.copy(ob[:, :], po[:, :])
            else:
                V.tensor_scalar_add(ob[:, :], po[:, :], 0.0)
            DMA2.dma_start(out_d[s, :, n2 * 512:(n2 + 1) * 512], ob[:, :])


_NC = None


def _get_nc():
    global _NC
    if _NC is None:
        _NC = build_nc()
    return _NC


def shard_inputs(attn_weights, hidden_states, metric, text_emb):
    """Host-side shard: slice CLS attention row, transpose to token-major
    column layout, cast hidden to bf16, split batch across cores."""
    B = attn_weights.shape[0]
    per = B // N_CORES
    attn_row = np.ascontiguousarray(attn_weights[:, :, 0, :], dtype=np.float32)
    h_b = np.asarray(hidden_states, np.float32).astype(NPBF16)
    met = np.asarray(metric, np.float32)
    consts = _consts()
    in_maps = []
    for c in range(N_CORES):
        sl = slice(c * per, (c + 1) * per)
        # attnT: [4,16,577] -> [577,4,16] -> pad 640 -> [128, 5, 64]
        at = attn_row[sl].transpose(2, 0, 1)                   # [577, 4, 16]
        atp = np.zeros((LPAD, per, NH), np.float32)
        atp[:L] = at
        atT = np.ascontiguousarray(
            atp.reshape(5, 128, per * NH).transpose(1, 0, 2))  # [128, 5, 64]
        # metricT: [4,577,64] -> [577,4,64] -> pad 640 -> [128, 5, 4, 64]
        mtc = met[sl].transpose(1, 0, 2)                       # [577, 4, 64]
        mtp = np.zeros((LPAD, per, CK), np.float32)
        mtp[:L] = mtc
        mtT = np.ascontiguousarray(
            mtp.reshape(5, 128, per, CK).transpose(1, 0, 2, 3))
        m = {
            "attnT": atT,
            "metricT": mtT,
            "text": np.ascontiguousarray(text_emb[sl]).astype(np.float32),
            "hidb": np.ascontiguousarray(h_b[sl]),
        }
        m.update(consts)
        in_maps.append(m)
    return in_maps


def kernel(attn_weights, hidden_states, metric, text_emb):
    nc = _get_nc()
    in_maps = shard_inputs(attn_weights, hidden_states, metric, text_emb)
    res = run_bass_kernel_spmd(nc, in_maps, core_ids=list(range(N_CORES)))
    out = np.concatenate([r["out"] for r in res.results], axis=0)
    return out.astype(np.float32)
